# revision 6
# baseline (speedup 1.0000x reference)
"""Block-circulant linear layer on 8 Trainium2 NeuronCores.

Math: y[n, (j,b)] = sum_i circconv(x[n,i,:], c[j,i,:])[b] + bias.
Factorized via packed-real-FFT (halfcomplex, 128 slots of (re,im)):

  stage A (rfft):    t1 = F_pack^T @ x^T     per in-block i, block-major out
  permute A->B:      slot-major regroup (DMA row shuffle)
  stage B (mixing):  t2 = W2_g^T @ t1p       block-diagonal per slot-group g
  permute B->C:      block-major regroup (DMA row shuffle)
  stage C (irfft):   y[tok, b] = t2p_tile^T @ G   (token-major output) + bias

All matmuls run as fp32r (full-rate fp32) with N=256 moving columns.
Sharding: data-parallel, 1024 tokens per core; weights replicated.
Host preprocessing: transpose+chunk x shards, build F/W2/G/bias layouts.
"""

import numpy as np

BLOCK = 256
NB = 16          # in/out blocks
NSLOT = 128      # frequency slots (halfcomplex pairs)
N_CORES = 8
TOK_PER_CORE = 1024
CHUNK = 256      # tokens per pipeline chunk
N_CHUNKS = TOK_PER_CORE // CHUNK
IN_F = NB * BLOCK  # 4096


def _build_weights(c: np.ndarray):
    """Host-side weight construction (float64 for accuracy, cast to f32)."""
    B, K = BLOCK, NSLOT
    b = np.arange(B)
    k = np.arange(K)
    theta = 2 * np.pi * np.outer(b, k) / B
    F_re = np.cos(theta)
    F_im = -np.sin(theta)
    F_im[:, 0] = (-1.0) ** b            # Nyquist column in the c=1 half, k=0
    F_pack = np.concatenate([F_re, F_im], axis=1)   # [256 b, 256 (c,k)]

    G_re = np.zeros((K, B))
    G_im = np.zeros((K, B))
    kk = np.arange(1, K)
    th = 2 * np.pi * np.outer(kk, b) / B
    G_re[1:] = 2.0 * np.cos(th) / B
    G_re[0] = 1.0 / B
    G_im[1:] = -2.0 * np.sin(th) / B
    G_im[0] = ((-1.0) ** b) / B
    G_pack = np.stack([G_re, G_im], axis=0)          # [2, 128 k, 256 b]

    Cf = np.fft.rfft(c.astype(np.float64), axis=-1)  # [j, i, 129]
    A = Cf.real
    Bm = Cf.imag
    W2 = np.zeros((32, 128, 128))
    for g in range(32):
        for s in range(4):
            ks = 4 * g + s
            blk = np.zeros((32, 32))                 # rows (c,i) -> cols (c',j)
            if ks == 0:
                blk[0:16, 0:16] = A[:, :, 0].T
                blk[16:32, 16:32] = A[:, :, 128].T
            else:
                a = A[:, :, ks].T
                bb = Bm[:, :, ks].T
                blk[0:16, 0:16] = a
                blk[16:32, 0:16] = -bb
                blk[0:16, 16:32] = bb
                blk[16:32, 16:32] = a
            W2[g, 32 * s:32 * s + 32, 32 * s:32 * s + 32] = blk

    f_host = (
        F_pack.reshape(2, 128, 2, 128).transpose(1, 0, 2, 3).reshape(128, 512)
    )  # [p=b_local, bh*256 + ch*128 + k]
    w2_host = W2.transpose(1, 0, 2).reshape(128, 32 * 128)   # [p, 128g + m]
    g_host = G_pack.transpose(1, 0, 2).reshape(128, 512)     # [k, ch*256 + b]
    return (
        f_host.astype(np.float32),
        w2_host.astype(np.float32),
        g_host.astype(np.float32),
    )


_NC_CACHE = {}


def _build_module():
    """Build + compile the per-core Bass module (cached)."""
    if "nc" in _NC_CACHE:
        return _NC_CACHE["nc"]

    import concourse.bass as bass  # noqa: F401
    import concourse.mybir as mybir
    import concourse.tile as tile
    from concourse import bacc

    f32 = mybir.dt.float32
    f32r = mybir.dt.float32r

    nc = bacc.Bacc("TRN2", target_bir_lowering=False, debug=False)

    xt_d = nc.dram_tensor(
        "xt", [N_CHUNKS, 128, 32, CHUNK], f32r, kind="ExternalInput"
    )
    f_d = nc.dram_tensor("fw", [128, 512], f32r, kind="ExternalInput")
    w2_d = nc.dram_tensor("w2", [128, 4096], f32r, kind="ExternalInput")
    g_d = nc.dram_tensor("gw", [128, 512], f32r, kind="ExternalInput")
    bias_d = nc.dram_tensor("biasr", [128, IN_F], f32, kind="ExternalInput")
    y_d = nc.dram_tensor("y", [TOK_PER_CORE, IN_F], f32, kind="ExternalOutput")

    with tile.TileContext(nc) as tc:
        with (
            tc.tile_pool(name="wpool", bufs=1) as wpool,
            tc.tile_pool(name="big", bufs=4) as big,
            tc.tile_pool(name="psA", bufs=2, space="PSUM") as psA,
            tc.tile_pool(name="psB", bufs=2, space="PSUM") as psB,
            tc.tile_pool(name="psC", bufs=2, space="PSUM") as psC,
        ):
            f_sb = wpool.tile([128, 512], f32r, tag="fw")
            w2_sb = wpool.tile([128, 4096], f32r, tag="w2")
            g_sb = wpool.tile([128, 512], f32r, tag="gw")
            bias_sb = wpool.tile([128, IN_F], f32, tag="bias")
            nc.sync.dma_start(out=f_sb[:], in_=f_d[:])
            nc.sync.dma_start(out=w2_sb[:], in_=w2_d[:])
            nc.sync.dma_start(out=g_sb[:], in_=g_d[:])
            nc.sync.dma_start(out=bias_sb[:], in_=bias_d[:])

            for ci in range(N_CHUNKS):
                # ---- load x^T chunk: [128 p, 32 f, 256 t] ----
                xts = big.tile([128, 8192], f32r, tag="big")
                nc.sync.dma_start(
                    out=xts[:].rearrange("p (f t) -> p f t", f=32),
                    in_=xt_d[ci],
                )

                # ---- stage A: rfft per in-block ----
                # t1 chunk q1 = 16*ch + i holds rows (k) for component ch of block i
                t1 = big.tile([128, 8192], f32r, tag="big")
                for i in range(NB):
                    for ch in range(2):
                        ps = psA.tile([128, CHUNK], f32, tag="psA")
                        for bh in range(2):
                            nc.tensor.matmul(
                                ps[:],
                                f_sb[:, bh * 256 + ch * 128: bh * 256 + ch * 128 + 128],
                                xts[:, (2 * i + bh) * 256: (2 * i + bh) * 256 + 256],
                                start=(bh == 0),
                                stop=(bh == 1),
                            )
                        q1 = 16 * ch + i
                        if i % 2 == 0:
                            nc.vector.tensor_copy(t1[:, q1 * 256: q1 * 256 + 256], ps[:])
                        else:
                            nc.scalar.copy(t1[:, q1 * 256: q1 * 256 + 256], ps[:])

                # ---- permute A->B: slot-major regroup ----
                # t1p[32s+16c+i, g, t] = t1[k=4g+s, 16c+i, t]
                t1p = big.tile([128, 8192], f32r, tag="big")
                t1v = t1[:].rearrange("p (q m) -> p q m", m=CHUNK)
                for g in range(32):
                    nc.sync.dma_start(
                        out=t1p[:, g * 256: g * 256 + 256],
                        in_=t1v[4 * g: 4 * g + 4],
                    )

                # ---- stage B: per-slot complex mixing (block-diagonal) ----
                t2 = big.tile([128, 8192], f32r, tag="big")
                for g in range(32):
                    ps = psB.tile([128, CHUNK], f32, tag="psB")
                    nc.tensor.matmul(
                        ps[:],
                        w2_sb[:, g * 128: g * 128 + 128],
                        t1p[:, g * 256: g * 256 + 256],
                        start=True,
                        stop=True,
                    )
                    if g % 2 == 0:
                        nc.vector.tensor_copy(t2[:, g * 256: g * 256 + 256], ps[:])
                    else:
                        nc.scalar.copy(t2[:, g * 256: g * 256 + 256], ps[:])

                # ---- permute B->C: block-major regroup ----
                # t2p[k=4g+s, 16c+j, t] = t2[32s+16c+j, g, t]
                t2p = big.tile([128, 8192], f32r, tag="big")
                t2pv = t2p[:].rearrange("p (q m) -> p q m", m=CHUNK)
                for g in range(32):
                    nc.sync.dma_start(
                        out=t2pv[4 * g: 4 * g + 4],
                        in_=t2[:, g * 256: g * 256 + 256],
                    )

                # ---- stage C: irfft, token-major output ----
                ysb = big.tile([128, 8192], f32, tag="big")
                for j in range(NB):
                    for tsub in range(2):
                        ps = psC.tile([128, BLOCK], f32, tag="psC")
                        for ch in range(2):
                            q4 = 16 * ch + j
                            nc.tensor.matmul(
                                ps[:],
                                t2p[:, q4 * 256 + 128 * tsub: q4 * 256 + 128 * tsub + 128],
                                g_sb[:, ch * 256: ch * 256 + 256],
                                start=(ch == 0),
                                stop=(ch == 1),
                            )
                        nc.vector.tensor_add(
                            ysb[:, tsub * 4096 + j * 256: tsub * 4096 + j * 256 + 256],
                            ps[:],
                            bias_sb[:, j * 256: j * 256 + 256],
                        )

                # ---- store y chunk ----
                nc.sync.dma_start(
                    out=y_d[ci * 256: ci * 256 + 256, :].rearrange(
                        "(s p) o -> p s o", p=128
                    ),
                    in_=ysb[:].rearrange("p (s o) -> p s o", s=2),
                )

    nc.compile()
    _NC_CACHE["nc"] = nc
    return nc


def kernel(x: np.ndarray, c: np.ndarray, bias: np.ndarray) -> np.ndarray:
    from concourse.bass_utils import run_bass_kernel_spmd

    batch, seq, in_f = x.shape
    n_tok = batch * seq
    xf = np.ascontiguousarray(x.reshape(n_tok, in_f).astype(np.float32))

    f_host, w2_host, g_host = _build_weights(np.asarray(c, dtype=np.float32))
    bias_host = np.ascontiguousarray(
        np.broadcast_to(np.asarray(bias, dtype=np.float32), (128, IN_F))
    )

    nc = _build_module()

    in_maps = []
    for core in range(N_CORES):
        shard = xf[core * TOK_PER_CORE:(core + 1) * TOK_PER_CORE]  # [1024, 4096]
        # xt[ci, p, f, t] = shard[ci*256 + t, 128*f + p]
        xt = np.ascontiguousarray(
            shard.reshape(N_CHUNKS, CHUNK, 32, 128).transpose(0, 3, 2, 1)
        )
        in_maps.append(
            {
                "xt": xt,
                "fw": f_host,
                "w2": w2_host,
                "gw": g_host,
                "biasr": bias_host,
            }
        )

    res = run_bass_kernel_spmd(nc, in_maps, core_ids=list(range(N_CORES)))
    y = np.concatenate([r["y"] for r in res.results], axis=0)  # [8192, 4096]
    return y.reshape(batch, seq, in_f).astype(x.dtype)
